# revision 4
# baseline (speedup 1.0000x reference)
"""LowRankAttention Trainium2 kernel v2 (Bass/Tile), data-parallel over 8 cores.

Math per batch b (one batch per core):
    Q = q @ Wq^T, K = k @ Wk^T, V = v @ Wv^T          (rank projections, R=256)
    A = softmax(Q K^T / sqrt(R))                       (softmax over keys j)
    out = (A @ V) @ Wo^T

v2 changes vs baseline:
  - q/k/v and weights cast to bf16 on host: input HBM traffic halves
    (27->13.6 MiB per core); all projection matmuls run bf16 (same PE speed).
  - Row sums of E=exp(A) no longer use ones-row matmuls on PE (was 13.6us of
    PE time).  Instead E^T tiles are accumulated over j-tiles on DVE (even jt)
    and GpSimd (odd jt) into two [128,512] accumulators; 8 single-row matmuls
    per chunk then give the per-query sums TRANSPOSED [i,1] directly -- which
    also kills the DRAM round-trip transpose of the old kernel.
  - V projection is interleaved into chunk 0's attention steps (PE no longer
    stalls waiting for v to stream in).
  - Output-projection groups of chunk ic are interleaved into chunk ic+1's
    attention steps; per-query 1/rowsum scaling alternates ACT/DVE.
  - DMA issue order matches consumption: wk, k, wq, q0, wv, v, q1-3, wo.
"""

import numpy as np

import concourse.bacc as bacc
import concourse.mybir as mybir
import concourse.tile as tile
from concourse import bass_utils

F32 = mybir.dt.float32
F32R = mybir.dt.float32r
BF16 = mybir.dt.bfloat16
AF = mybir.ActivationFunctionType
ALU = mybir.AluOpType

DIM, RANK, B, S = 1024, 256, 8, 2048
P = 128
NC = 512                      # moving-operand / psum free chunk
DT = DIM // P                 # 8  d-tiles
RT = RANK // P                # 2  r-tiles
SC = S // NC                  # 4  s-chunks (i-chunks and j-quarters)
JT = S // P                   # 16 j-tiles
IT = NC // P                  # 4  i-tiles per chunk
DC = DIM // NC                # 2  d-chunks of output
SCALE = 1.0 / np.sqrt(np.float32(RANK))
LAG = 2


def build_program(reps: int = 1, unroll: int = 1):
    nc = bacc.Bacc("TRN2", target_bir_lowering=False, debug=False)

    qT = nc.dram_tensor("qT", [DIM, S], BF16, kind="ExternalInput")
    kT = nc.dram_tensor("kT", [DIM, S], BF16, kind="ExternalInput")
    vT = nc.dram_tensor("vT", [DIM, S], BF16, kind="ExternalInput")
    wqT = nc.dram_tensor("wqT", [DIM, RANK], BF16, kind="ExternalInput")
    wkT = nc.dram_tensor("wkT", [DIM, RANK], BF16, kind="ExternalInput")
    wvT = nc.dram_tensor("wvT", [DIM, RANK], BF16, kind="ExternalInput")
    woT = nc.dram_tensor("woT", [RANK, DIM], BF16, kind="ExternalInput")
    out = nc.dram_tensor("out", [S, DIM], F32, kind="ExternalOutput")

    with tile.TileContext(nc) as tc:
        with tc.tile_pool(name="w", bufs=1) as wpool, \
             tc.tile_pool(name="inp", bufs=20) as inpool, \
             tc.tile_pool(name="inq", bufs=4) as qpool, \
             tc.tile_pool(name="per", bufs=1) as perpool, \
             tc.tile_pool(name="qt", bufs=2) as qtpool, \
             tc.tile_pool(name="et", bufs=6) as etpool, \
             tc.tile_pool(name="acc", bufs=4) as accpool, \
             tc.tile_pool(name="av", bufs=4) as avpool, \
             tc.tile_pool(name="o", bufs=3) as opool, \
             tc.tile_pool(name="sm", bufs=2) as smpool, \
             tc.tile_pool(name="ps", bufs=3, space="PSUM") as pspool, \
             tc.tile_pool(name="pso", bufs=3, space="PSUM") as psopool, \
             tc.tile_pool(name="psav", bufs=2, space="PSUM") as psavpool:

            def body(_i=None):
                # ---- weight tiles ----
                wk_t = wpool.tile([P, DT, RANK], BF16, tag="wk", name="wk_t")
                wq_t = wpool.tile([P, DT, RANK], BF16, tag="wq", name="wq_t")
                wv_t = wpool.tile([P, DT, RANK], BF16, tag="wv", name="wv_t")
                wo_t = wpool.tile([P, RT, DIM], BF16, tag="wo", name="wo_t")
                ones_f = wpool.tile([P, 1], F32, tag="onesf", name="ones_f")
                nc.vector.memset(ones_f[:], 1.0)
                ones = wpool.tile([P, 1], BF16, tag="ones", name="ones")
                nc.vector.tensor_copy(ones[:], ones_f[:])

                # ---- DMA issue order == consumption order; few, large DMAs
                # (each DMA costs ~0.6us of HWDGE config time regardless of
                # size, so granularity is halves/whole-tensors, not quarters)
                H = S // 2
                nc.sync.dma_start(wk_t[:], wkT.ap().rearrange("(dt p) r -> p dt r", p=P))
                ktiles = {}
                for h in range(2):
                    for dt in range(DT):
                        t = inpool.tile([P, H], BF16, tag="inKV", name=f"k_{dt}_{h}")
                        nc.sync.dma_start(
                            t[:], kT.ap()[dt * P:(dt + 1) * P, h * H:(h + 1) * H])
                        ktiles[(dt, h)] = t
                nc.sync.dma_start(wq_t[:], wqT.ap().rearrange("(dt p) r -> p dt r", p=P))
                qtiles = {}

                def load_q(ic):
                    t = qpool.tile([P, DT, NC], BF16, tag="inQ", name=f"q_{ic}")
                    nc.sync.dma_start(
                        t[:], qT.ap()[:, ic * NC:(ic + 1) * NC]
                        .rearrange("(dt p) c -> p dt c", p=P))
                    qtiles[ic] = t

                load_q(0)
                nc.sync.dma_start(wv_t[:], wvT.ap().rearrange("(dt p) r -> p dt r", p=P))
                vtiles = {}
                for h in range(2):
                    for dt in range(DT):
                        t = inpool.tile([P, H], BF16, tag="inKV", name=f"v_{dt}_{h}")
                        nc.sync.dma_start(
                            t[:], vT.ap()[dt * P:(dt + 1) * P, h * H:(h + 1) * H])
                        vtiles[(dt, h)] = t
                for ic in range(1, SC):
                    load_q(ic)
                nc.sync.dma_start(wo_t[:], woT.ap().rearrange("(rt p) d -> p rt d", p=P))

                # ---- persistent projection outputs ----
                KT_t = perpool.tile([P, RT, S], F32R, tag="KT", name="KT_t")   # [r_p, rt, j]
                V_t = perpool.tile([P, JT, RANK], F32R, tag="V", name="V_t")   # [j_p, jt, r]

                # ---- K projection (per s-chunk, straight off the DMA) ----
                for sc in range(SC):
                    h, o = sc // 2, (sc % 2) * NC
                    for rt in range(RT):
                        ps = pspool.tile([P, NC], F32, tag="ps", name="ps_projk")
                        for dt in range(DT):
                            nc.tensor.matmul(ps[:], wk_t[:, dt, rt * P:(rt + 1) * P],
                                             ktiles[(dt, h)][:, o:o + NC],
                                             start=(dt == 0), stop=(dt == DT - 1))
                        nc.scalar.copy(KT_t[:, rt, sc * NC:(sc + 1) * NC], ps[:])

                def vproj(jt):
                    h, o = jt // 8, (jt % 8) * P
                    ps = psopool.tile([P, RANK], F32, tag="pso", name="ps_v")
                    for dt in range(DT):
                        nc.tensor.matmul(ps[:], vtiles[(dt, h)][:, o:o + P], wv_t[:, dt, :],
                                         start=(dt == 0), stop=(dt == DT - 1))
                    nc.scalar.copy(V_t[:, jt, :], ps[:])

                def qproj(ic):
                    qt = qtpool.tile([P, RT, NC], F32R, tag="qt", name="qt_t")
                    for rt in range(RT):
                        ps = pspool.tile([P, NC], F32, tag="ps", name="ps_projq")
                        for dt in range(DT):
                            nc.tensor.matmul(ps[:], wq_t[:, dt, rt * P:(rt + 1) * P],
                                             qtiles[ic][:, dt, :],
                                             start=(dt == 0), stop=(dt == DT - 1))
                        nc.scalar.copy(qt[:, rt, :], ps[:])
                    return qt

                # out-projection group g of a finished chunk: 2 matmuls + scale
                def outgroup(ctx, g):
                    it, dc = g // DC, g % DC
                    avt, inv, ic = ctx
                    ps = psopool.tile([P, NC], F32, tag="pso", name="ps_o")
                    for rt in range(RT):
                        nc.tensor.matmul(ps[:], avt[rt][:, it * P:(it + 1) * P],
                                         wo_t[:, rt, dc * NC:(dc + 1) * NC],
                                         start=(rt == 0), stop=(rt == RT - 1))
                    ot = opool.tile([P, NC], F32, tag="out", name="ot")
                    if g % 2 == 0:
                        nc.scalar.mul(ot[:], ps[:], inv[:, it:it + 1])
                    else:
                        nc.vector.tensor_scalar_mul(ot[:], ps[:], inv[:, it:it + 1])
                    i0 = ic * NC + it * P
                    nc.sync.dma_start(out.ap()[i0:i0 + P, dc * NC:(dc + 1) * NC], ot[:])

                # ---- chunk loop ----
                prev_ctx = None
                qt = qproj(0)
                for ic in range(SC):
                    accA = accpool.tile([P, NC], BF16, tag="acc", name="accA")
                    accB = accpool.tile([P, NC], BF16, tag="acc", name="accB")
                    av_ps = [psavpool.tile([P, NC], F32, tag="av", name=f"av_{rt}")
                             for rt in range(RT)]
                    ets = {}

                    def at_step(jt, qt=qt, accA=accA, accB=accB, ets=ets):
                        ps = pspool.tile([P, NC], F32, tag="ps", name="ps_at")
                        for rt in range(RT):
                            nc.tensor.matmul(ps[:], KT_t[:, rt, jt * P:(jt + 1) * P],
                                             qt[:, rt, :],
                                             start=(rt == 0), stop=(rt == RT - 1))
                        et = etpool.tile([P, NC], F32R, tag="et", name="et")
                        nc.scalar.activation(et[:], ps[:], AF.Exp, scale=float(SCALE))
                        ets[jt] = et
                        # accumulate E^T tiles for the row sums (partition dim
                        # reduction happens later via 1-row matmuls); odd jt on
                        # DVE so the last tile's add is on the faster engine
                        eng, acc = (nc.vector, accA) if jt % 2 == 1 else (nc.gpsimd, accB)
                        if jt < 2:
                            eng.tensor_copy(acc[:], et[:])
                        else:
                            eng.tensor_tensor(acc[:], acc[:], et[:], op=ALU.add)

                    def ev_step(jt, av_ps=av_ps, ets=ets):
                        et = ets.pop(jt)
                        for rt in range(RT):
                            nc.tensor.matmul(av_ps[rt][:], V_t[:, jt, rt * P:(rt + 1) * P],
                                             et[:],
                                             start=(jt == 0), stop=(jt == JT - 1))

                    # on the last chunk hold back two of the previous chunk's
                    # output groups to keep PE busy over the accumulator adds
                    ng = DC * IT if ic < SC - 1 else DC * IT - 2
                    for jt in range(JT):
                        at_step(jt)
                        if ic == 0:
                            vproj(jt)
                        elif jt < ng:
                            outgroup(prev_ctx, jt)
                        if jt >= LAG:
                            ev_step(jt - LAG)
                    for jt in range(JT - LAG, JT):
                        ev_step(jt)

                    # avt copies first (DVE + ACT), then next chunk's Q
                    # projection fills the PE while the accumulator adds and
                    # copies land, then the tiny transposed row-sum matmuls
                    avt = []
                    for rt in range(RT):
                        t = avpool.tile([P, NC], BF16, tag="avt", name=f"avt_{rt}")
                        if rt == 0:
                            nc.vector.tensor_copy(t[:], av_ps[rt][:])
                        else:
                            nc.scalar.copy(t[:], av_ps[rt][:])
                        avt.append(t)
                    if ic + 1 < SC:
                        qt = qproj(ic + 1)
                    else:
                        for g in range(ng, DC * IT):
                            outgroup(prev_ctx, g)

                    sums_ps = psopool.tile([P, IT], F32, tag="pso", name="sums_ps")
                    for b in range(IT):
                        nc.tensor.matmul(sums_ps[:, b:b + 1],
                                         accA[:, b * P:(b + 1) * P], ones[:],
                                         start=True, stop=False)
                        nc.tensor.matmul(sums_ps[:, b:b + 1],
                                         accB[:, b * P:(b + 1) * P], ones[:],
                                         start=False, stop=True)
                    inv = smpool.tile([P, IT], F32, tag="inv", name="inv")
                    nc.vector.reciprocal(inv[:], sums_ps[:])

                    prev_ctx = (avt, inv, {}, ic)

                # drain the last chunk's output projection
                for g in range(DC * IT):
                    outgroup(prev_ctx, g)

            if reps == 1:
                for _ in range(unroll):
                    body()
            elif reps % 2 == 0:
                # pair of bodies per hardware-loop iteration: adjacent
                # iterations overlap across the drain-free pair boundary
                with tc.For_i(0, reps // 2, 1) as i:
                    body(i)
                    body(i)
            else:
                with tc.For_i(0, reps, 1) as i:
                    body(i)

    nc.compile()
    return nc


_CACHE = {}


def _get_program():
    if "nc" not in _CACHE:
        _CACHE["nc"] = build_program(reps=1)
    return _CACHE["nc"]


def prep_inputs(q, k, v, Wq, Wk, Wv, Wo):
    """Host-side layout/dtype prep: transpose so the contraction dim (D) lands
    on SBUF partitions, cast to bf16; one batch per core."""
    import ml_dtypes
    bf16 = ml_dtypes.bfloat16
    qT = np.ascontiguousarray(np.asarray(q, np.float32).transpose(0, 2, 1)).astype(bf16)
    kT = np.ascontiguousarray(np.asarray(k, np.float32).transpose(0, 2, 1)).astype(bf16)
    vT = np.ascontiguousarray(np.asarray(v, np.float32).transpose(0, 2, 1)).astype(bf16)
    wqT = np.ascontiguousarray(np.asarray(Wq, np.float32).T).astype(bf16)
    wkT = np.ascontiguousarray(np.asarray(Wk, np.float32).T).astype(bf16)
    wvT = np.ascontiguousarray(np.asarray(Wv, np.float32).T).astype(bf16)
    woT = np.ascontiguousarray(np.asarray(Wo, np.float32).T).astype(bf16)
    return [{"qT": qT[c], "kT": kT[c], "vT": vT[c],
             "wqT": wqT, "wkT": wkT, "wvT": wvT, "woT": woT}
            for c in range(B)]


def kernel(q, k, v, Wq, Wk, Wv, Wo):
    nc = _get_program()
    in_maps = prep_inputs(q, k, v, Wq, Wk, Wv, Wo)
    res = bass_utils.run_bass_kernel_spmd(nc, in_maps, core_ids=list(range(B)))
    return np.stack([res.results[c]["out"] for c in range(B)], axis=0)


# revision 5
# speedup vs baseline: 1.1023x; 1.1023x over previous
"""LowRankAttention Trainium2 kernel v2 (Bass/Tile), data-parallel over 8 cores.

Math per batch b (one batch per core):
    Q = q @ Wq^T, K = k @ Wk^T, V = v @ Wv^T          (rank projections, R=256)
    A = softmax(Q K^T / sqrt(R))                       (softmax over keys j)
    out = (A @ V) @ Wo^T

v2 changes vs baseline:
  - q/k/v and weights cast to bf16 on host: input HBM traffic halves
    (27->13.6 MiB per core); all projection matmuls run bf16 (same PE speed).
  - Row sums of E=exp(A) no longer use ones-row matmuls on PE (was 13.6us of
    PE time).  Instead E^T tiles are accumulated over j-tiles on DVE (even jt)
    and GpSimd (odd jt) into two [128,512] accumulators; 8 single-row matmuls
    per chunk then give the per-query sums TRANSPOSED [i,1] directly -- which
    also kills the DRAM round-trip transpose of the old kernel.
  - V projection is interleaved into chunk 0's attention steps (PE no longer
    stalls waiting for v to stream in).
  - Output-projection groups of chunk ic are interleaved into chunk ic+1's
    attention steps; per-query 1/rowsum scaling alternates ACT/DVE.
  - DMA issue order matches consumption: wk, k, wq, q0, wv, v, q1-3, wo.
"""

import numpy as np

import concourse.bacc as bacc
import concourse.mybir as mybir
import concourse.tile as tile
from concourse import bass_utils

F32 = mybir.dt.float32
F32R = mybir.dt.float32r
BF16 = mybir.dt.bfloat16
AF = mybir.ActivationFunctionType
ALU = mybir.AluOpType

DIM, RANK, B, S = 1024, 256, 8, 2048
P = 128
NC = 512                      # moving-operand / psum free chunk
DT = DIM // P                 # 8  d-tiles
RT = RANK // P                # 2  r-tiles
SC = S // NC                  # 4  s-chunks (i-chunks and j-quarters)
JT = S // P                   # 16 j-tiles
IT = NC // P                  # 4  i-tiles per chunk
DC = DIM // NC                # 2  d-chunks of output
SCALE = 1.0 / np.sqrt(np.float32(RANK))
LAG = 2


def build_program(reps: int = 1, unroll: int = 1):
    nc = bacc.Bacc("TRN2", target_bir_lowering=False, debug=False)

    qT = nc.dram_tensor("qT", [DIM, S], BF16, kind="ExternalInput")
    kT = nc.dram_tensor("kT", [DIM, S], BF16, kind="ExternalInput")
    vT = nc.dram_tensor("vT", [DIM, S], BF16, kind="ExternalInput")
    wqT = nc.dram_tensor("wqT", [DIM, RANK], BF16, kind="ExternalInput")
    wkT = nc.dram_tensor("wkT", [DIM, RANK], BF16, kind="ExternalInput")
    wvT = nc.dram_tensor("wvT", [DIM, RANK], BF16, kind="ExternalInput")
    woT = nc.dram_tensor("woT", [RANK, DIM], BF16, kind="ExternalInput")
    out = nc.dram_tensor("out", [S, DIM], F32, kind="ExternalOutput")

    with tile.TileContext(nc) as tc:
        with tc.tile_pool(name="w", bufs=1) as wpool, \
             tc.tile_pool(name="inp", bufs=20) as inpool, \
             tc.tile_pool(name="inq", bufs=4) as qpool, \
             tc.tile_pool(name="per", bufs=1) as perpool, \
             tc.tile_pool(name="qt", bufs=2) as qtpool, \
             tc.tile_pool(name="et", bufs=6) as etpool, \
             tc.tile_pool(name="acc", bufs=4) as accpool, \
             tc.tile_pool(name="av", bufs=4) as avpool, \
             tc.tile_pool(name="o", bufs=3) as opool, \
             tc.tile_pool(name="sm", bufs=2) as smpool, \
             tc.tile_pool(name="ps", bufs=3, space="PSUM") as pspool, \
             tc.tile_pool(name="pso", bufs=3, space="PSUM") as psopool, \
             tc.tile_pool(name="psav", bufs=2, space="PSUM") as psavpool:

            def body(_i=None):
                # ---- weight tiles ----
                wk_t = wpool.tile([P, DT, RANK], BF16, tag="wk", name="wk_t")
                wq_t = wpool.tile([P, DT, RANK], BF16, tag="wq", name="wq_t")
                wv_t = wpool.tile([P, DT, RANK], BF16, tag="wv", name="wv_t")
                wo_t = wpool.tile([P, RT, DIM], BF16, tag="wo", name="wo_t")
                ones_f = wpool.tile([P, 1], F32, tag="onesf", name="ones_f")
                nc.vector.memset(ones_f[:], 1.0)
                ones = wpool.tile([P, 1], BF16, tag="ones", name="ones")
                nc.vector.tensor_copy(ones[:], ones_f[:])

                # ---- DMA issue order == consumption order; few, large DMAs
                # (each DMA costs ~0.6us of HWDGE config time regardless of
                # size, so granularity is halves/whole-tensors, not quarters)
                H = S // 2
                nc.sync.dma_start(wk_t[:], wkT.ap().rearrange("(dt p) r -> p dt r", p=P))
                ktiles = {}
                for h in range(2):
                    for dt in range(DT):
                        t = inpool.tile([P, H], BF16, tag="inKV", name=f"k_{dt}_{h}")
                        nc.sync.dma_start(
                            t[:], kT.ap()[dt * P:(dt + 1) * P, h * H:(h + 1) * H])
                        ktiles[(dt, h)] = t
                nc.sync.dma_start(wq_t[:], wqT.ap().rearrange("(dt p) r -> p dt r", p=P))
                qtiles = {}

                def load_q(ic):
                    t = qpool.tile([P, DT, NC], BF16, tag="inQ", name=f"q_{ic}")
                    nc.sync.dma_start(
                        t[:], qT.ap()[:, ic * NC:(ic + 1) * NC]
                        .rearrange("(dt p) c -> p dt c", p=P))
                    qtiles[ic] = t

                load_q(0)
                nc.sync.dma_start(wv_t[:], wvT.ap().rearrange("(dt p) r -> p dt r", p=P))
                vtiles = {}
                for h in range(2):
                    for dt in range(DT):
                        t = inpool.tile([P, H], BF16, tag="inKV", name=f"v_{dt}_{h}")
                        nc.sync.dma_start(
                            t[:], vT.ap()[dt * P:(dt + 1) * P, h * H:(h + 1) * H])
                        vtiles[(dt, h)] = t
                for ic in range(1, SC):
                    load_q(ic)
                nc.sync.dma_start(wo_t[:], woT.ap().rearrange("(rt p) d -> p rt d", p=P))

                # ---- persistent projection outputs ----
                KT_t = perpool.tile([P, RT, S], F32R, tag="KT", name="KT_t")   # [r_p, rt, j]
                V_t = perpool.tile([P, JT, RANK], F32R, tag="V", name="V_t")   # [j_p, jt, r]

                # ---- K projection (per s-chunk, straight off the DMA) ----
                for sc in range(SC):
                    h, o = sc // 2, (sc % 2) * NC
                    for rt in range(RT):
                        ps = pspool.tile([P, NC], F32, tag="ps", name="ps_projk")
                        for dt in range(DT):
                            nc.tensor.matmul(ps[:], wk_t[:, dt, rt * P:(rt + 1) * P],
                                             ktiles[(dt, h)][:, o:o + NC],
                                             start=(dt == 0), stop=(dt == DT - 1))
                        nc.scalar.copy(KT_t[:, rt, sc * NC:(sc + 1) * NC], ps[:])

                def vproj(jt):
                    h, o = jt // 8, (jt % 8) * P
                    ps = psopool.tile([P, RANK], F32, tag="pso", name="ps_v")
                    for dt in range(DT):
                        nc.tensor.matmul(ps[:], vtiles[(dt, h)][:, o:o + P], wv_t[:, dt, :],
                                         start=(dt == 0), stop=(dt == DT - 1))
                    nc.scalar.copy(V_t[:, jt, :], ps[:])

                def qproj(ic):
                    qt = qtpool.tile([P, RT, NC], F32R, tag="qt", name="qt_t")
                    for rt in range(RT):
                        ps = pspool.tile([P, NC], F32, tag="ps", name="ps_projq")
                        for dt in range(DT):
                            nc.tensor.matmul(ps[:], wq_t[:, dt, rt * P:(rt + 1) * P],
                                             qtiles[ic][:, dt, :],
                                             start=(dt == 0), stop=(dt == DT - 1))
                        nc.scalar.copy(qt[:, rt, :], ps[:])
                    return qt

                # out-projection group g of a finished chunk: 2 matmuls + scale
                def outgroup(ctx, g):
                    it, dc = g // DC, g % DC
                    avt, inv, ic = ctx
                    ps = psopool.tile([P, NC], F32, tag="pso", name="ps_o")
                    for rt in range(RT):
                        nc.tensor.matmul(ps[:], avt[rt][:, it * P:(it + 1) * P],
                                         wo_t[:, rt, dc * NC:(dc + 1) * NC],
                                         start=(rt == 0), stop=(rt == RT - 1))
                    ot = opool.tile([P, NC], F32, tag="out", name="ot")
                    if g % 2 == 0:
                        nc.scalar.mul(ot[:], ps[:], inv[:, it:it + 1])
                    else:
                        nc.vector.tensor_scalar_mul(ot[:], ps[:], inv[:, it:it + 1])
                    i0 = ic * NC + it * P
                    nc.sync.dma_start(out.ap()[i0:i0 + P, dc * NC:(dc + 1) * NC], ot[:])

                # ---- chunk loop ----
                prev_ctx = None
                qt = qproj(0)
                for ic in range(SC):
                    accA = accpool.tile([P, NC], BF16, tag="acc", name="accA")
                    accB = accpool.tile([P, NC], BF16, tag="acc", name="accB")
                    av_ps = [psavpool.tile([P, NC], F32, tag="av", name=f"av_{rt}")
                             for rt in range(RT)]
                    ets = {}

                    def at_step(jt, qt=qt, accA=accA, accB=accB, ets=ets):
                        ps = pspool.tile([P, NC], F32, tag="ps", name="ps_at")
                        for rt in range(RT):
                            nc.tensor.matmul(ps[:], KT_t[:, rt, jt * P:(jt + 1) * P],
                                             qt[:, rt, :],
                                             start=(rt == 0), stop=(rt == RT - 1))
                        et = etpool.tile([P, NC], F32R, tag="et", name="et")
                        nc.scalar.activation(et[:], ps[:], AF.Exp, scale=float(SCALE))
                        ets[jt] = et
                        # accumulate E^T tiles for the row sums (partition dim
                        # reduction happens later via 1-row matmuls); odd jt on
                        # DVE so the last tile's add is on the faster engine
                        eng, acc = (nc.vector, accA) if jt % 2 == 1 else (nc.gpsimd, accB)
                        if jt < 2:
                            eng.tensor_copy(acc[:], et[:])
                        else:
                            eng.tensor_tensor(acc[:], acc[:], et[:], op=ALU.add)

                    def ev_step(jt, av_ps=av_ps, ets=ets):
                        et = ets.pop(jt)
                        for rt in range(RT):
                            nc.tensor.matmul(av_ps[rt][:], V_t[:, jt, rt * P:(rt + 1) * P],
                                             et[:],
                                             start=(jt == 0), stop=(jt == JT - 1))

                    # on the last chunk hold back two of the previous chunk's
                    # output groups to keep PE busy over the accumulator adds
                    ng = DC * IT if ic < SC - 1 else DC * IT - 2
                    for jt in range(JT):
                        at_step(jt)
                        if ic == 0:
                            vproj(jt)
                        elif jt < ng:
                            outgroup(prev_ctx, jt)
                        if jt >= LAG:
                            ev_step(jt - LAG)
                    for jt in range(JT - LAG, JT):
                        ev_step(jt)

                    # avt copies first (DVE + ACT), then next chunk's Q
                    # projection fills the PE while the accumulator adds and
                    # copies land, then the tiny transposed row-sum matmuls
                    avt = []
                    for rt in range(RT):
                        t = avpool.tile([P, NC], BF16, tag="avt", name=f"avt_{rt}")
                        if rt == 0:
                            nc.vector.tensor_copy(t[:], av_ps[rt][:])
                        else:
                            nc.scalar.copy(t[:], av_ps[rt][:])
                        avt.append(t)
                    if ic + 1 < SC:
                        qt = qproj(ic + 1)
                    else:
                        for g in range(ng, DC * IT):
                            outgroup(prev_ctx, g)

                    sums_ps = psopool.tile([P, IT], F32, tag="pso", name="sums_ps")
                    for b in range(IT):
                        nc.tensor.matmul(sums_ps[:, b:b + 1],
                                         accA[:, b * P:(b + 1) * P], ones[:],
                                         start=True, stop=False)
                        nc.tensor.matmul(sums_ps[:, b:b + 1],
                                         accB[:, b * P:(b + 1) * P], ones[:],
                                         start=False, stop=True)
                    inv = smpool.tile([P, IT], F32, tag="inv", name="inv")
                    nc.vector.reciprocal(inv[:], sums_ps[:])

                    prev_ctx = (avt, inv, {}, ic)

                # drain the last chunk's output projection
                for g in range(DC * IT):
                    outgroup(prev_ctx, g)

            if reps == 1:
                for _ in range(unroll):
                    body()
            else:
                with tc.For_i(0, reps, 1) as i:
                    body(i)

    nc.compile()
    return nc


_CACHE = {}


def _get_program():
    if "nc" not in _CACHE:
        _CACHE["nc"] = build_program(reps=1)
    return _CACHE["nc"]


def prep_inputs(q, k, v, Wq, Wk, Wv, Wo):
    """Host-side layout/dtype prep: transpose so the contraction dim (D) lands
    on SBUF partitions, cast to bf16; one batch per core."""
    import ml_dtypes
    bf16 = ml_dtypes.bfloat16
    qT = np.ascontiguousarray(np.asarray(q, np.float32).transpose(0, 2, 1)).astype(bf16)
    kT = np.ascontiguousarray(np.asarray(k, np.float32).transpose(0, 2, 1)).astype(bf16)
    vT = np.ascontiguousarray(np.asarray(v, np.float32).transpose(0, 2, 1)).astype(bf16)
    wqT = np.ascontiguousarray(np.asarray(Wq, np.float32).T).astype(bf16)
    wkT = np.ascontiguousarray(np.asarray(Wk, np.float32).T).astype(bf16)
    wvT = np.ascontiguousarray(np.asarray(Wv, np.float32).T).astype(bf16)
    woT = np.ascontiguousarray(np.asarray(Wo, np.float32).T).astype(bf16)
    return [{"qT": qT[c], "kT": kT[c], "vT": vT[c],
             "wqT": wqT, "wkT": wkT, "wvT": wvT, "woT": woT}
            for c in range(B)]


def kernel(q, k, v, Wq, Wk, Wv, Wo):
    nc = _get_program()
    in_maps = prep_inputs(q, k, v, Wq, Wk, Wv, Wo)
    res = bass_utils.run_bass_kernel_spmd(nc, in_maps, core_ids=list(range(B)))
    return np.stack([res.results[c]["out"] for c in range(B)], axis=0)
